# revision 16
# baseline (speedup 1.0000x reference)
"""Additive (Bahdanau) attention on 8 TRN2 NeuronCores.

scores[b,t,s] = softmax_s( sum_d v[d] * tanh(e1[b,s,d] + e2[b,t,d]) )  with mask
  e1 = enc @ We.T   [B,S,D]
  e2 = dec @ Wd.T   [B,T,D]

Sharding: pure data-parallel, core k handles batch b=k//2, t-half k%2
(128 t-rows each). No collectives.

Per-core device pipeline (all tensors laid out [d, *] so the v-reduction
runs on TensorE and the per-t bias-add rides free on ScalarE's ACTIVATE):
  e1T[d,s] = We @ encT     (TensorE, 4x8 accum matmuls)
  e2T[d,t] = Wd @ decT     (TensorE, 4x4)
  per t:  act_j = tanh(e1T_j + e2T_j[:,t])   (ScalarE ACTIVATE, bias=per-partition)
          row  += v_j.T @ act_j              (TensorE matvec -> PSUM [1,S])
  rows DMA-gathered 4-at-a-time (PSUM 4 banks -> scores_sb partitions; compute
  engines can't write partitions !=0/32/64/96, DMA can)
  softmax: +maskbias -> reduce_max(neg) -> Exp(bias=-max, accum_out=sums)
           -> recip -> scale
"""

import numpy as np

B, T, S, D = 4, 256, 512, 512
C = 2 * D
NCORES = 8
TLOC = 128  # t-rows per core
GROUP = 4  # score rows gathered per DMA (4 PSUM banks)

_CACHE = {}


def _build():
    import concourse.mybir as mybir
    from concourse import bacc
    from concourse.tile import TileContext

    f32 = mybir.dt.float32
    f16 = mybir.dt.float16
    AF = mybir.ActivationFunctionType

    nc = bacc.Bacc()
    encT_d = nc.declare_dram_parameter("encT", [C, S], f16, isOutput=False)
    decT_d = nc.declare_dram_parameter("decT", [D, TLOC], f16, isOutput=False)
    WeT_d = nc.declare_dram_parameter("WeT", [C, D], f16, isOutput=False)
    WdT_d = nc.declare_dram_parameter("WdT", [D, D], f16, isOutput=False)
    v4_d = nc.declare_dram_parameter("v4", [128, D // 128], f16, isOutput=False)
    mb_d = nc.declare_dram_parameter("maskbias", [1, GROUP * S], f32,
                                     isOutput=False)
    out_d = nc.declare_dram_parameter("out", [TLOC, S], f32, isOutput=True)

    ND = D // 128  # 4 d-tiles
    NC_ = C // 128  # 8 c-tiles

    with TileContext(nc) as tc:
        with tc.tile_pool(name="persist", bufs=1) as pp:
            dma = nc.default_dma_engine

            WeT_sb = []
            encT_sb = []
            for ci in range(NC_):
                tw = pp.tile([128, D], f16, tag=f"WeT{ci}")
                dma.dma_start(out=tw, in_=WeT_d[ci * 128:(ci + 1) * 128, :])
                WeT_sb.append(tw)
                te = pp.tile([128, S], f16, tag=f"encT{ci}")
                dma.dma_start(out=te, in_=encT_d[ci * 128:(ci + 1) * 128, :])
                encT_sb.append(te)
            WdT_sb = []
            decT_sb = []
            for di in range(ND):
                tw = pp.tile([128, D], f16, tag=f"WdT{di}")
                dma.dma_start(out=tw, in_=WdT_d[di * 128:(di + 1) * 128, :])
                WdT_sb.append(tw)
                td = pp.tile([128, TLOC], f16, tag=f"decT{di}")
                dma.dma_start(out=td, in_=decT_d[di * 128:(di + 1) * 128, :])
                decT_sb.append(td)
            v_sb = pp.tile([128, ND], f16, tag="v4")
            dma.dma_start(out=v_sb, in_=v4_d[:, :])

            # maskbias tiled GROUP times along free dim, single partition
            mb_sb = pp.tile([1, GROUP * S], f32, tag="mb_sb")
            dma.dma_start(out=mb_sb, in_=mb_d[:, :])

            # e1T[d,s] (4 tiles) and e2T[d,t] (4 tiles)
            e1T_sb = []
            e2T_sb = []
            with tc.tile_pool(name="mm_psum", bufs=2, space="PSUM") as mmp:
                for dj in range(ND):
                    ps = mmp.tile([128, S], f32, tag="pe1")
                    for ci in range(NC_):
                        nc.tensor.matmul(
                            ps,
                            WeT_sb[ci][:, dj * 128:(dj + 1) * 128],
                            encT_sb[ci],
                            start=(ci == 0),
                            stop=(ci == NC_ - 1),
                        )
                    t_ = pp.tile([128, S], f32, tag=f"e1T{dj}")
                    nc.vector.tensor_copy(t_, ps)
                    e1T_sb.append(t_)
                for ej in range(ND):
                    ps = mmp.tile([128, TLOC], f32, tag="pe2")
                    for di in range(ND):
                        nc.tensor.matmul(
                            ps,
                            WdT_sb[di][:, ej * 128:(ej + 1) * 128],
                            decT_sb[di],
                            start=(di == 0),
                            stop=(di == ND - 1),
                        )
                    t_ = pp.tile([128, TLOC], f32, tag=f"e2T{ej}")
                    nc.vector.tensor_copy(t_, ps)
                    e2T_sb.append(t_)

            scores_sb = pp.tile([TLOC, S], f32, tag="scores")
            with (
                tc.tile_pool(name="act", bufs=6) as ap_,
                tc.tile_pool(name="stage", bufs=3) as sp,
                tc.tile_pool(name="row_psum", bufs=2, space="PSUM") as rp,
            ):
                for g in range(TLOC // GROUP):
                    rowg = rp.tile([1, GROUP, S], f32, tag="rowg")
                    for i in range(GROUP):
                        t = g * GROUP + i
                        for j in range(ND):
                            act = ap_.tile([128, S], f16, tag="act")
                            nc.scalar.activation(
                                out=act,
                                in_=e1T_sb[j],
                                func=AF.Tanh,
                                bias=e2T_sb[j][:, t:t + 1],
                                scale=1.0,
                            )
                            nc.tensor.matmul(
                                rowg[0:1, i, :], v_sb[:, j:j + 1], act,
                                start=(j == 0), stop=(j == ND - 1),
                            )
                    # 4 PSUM banks -> one wide DVE add (+maskbias) -> SBUF row,
                    # then DMA re-spreads the row onto partitions 4g..4g+3
                    stage = sp.tile([1, GROUP * S], f32, tag="stage")
                    nc.vector.tensor_tensor(
                        stage, rowg[0:1, :, :], mb_sb, op=mybir.AluOpType.add
                    )
                    dma.dma_start(
                        out=scores_sb[g * GROUP:(g + 1) * GROUP, :],
                        in_=stage.rearrange("p (g s) -> p g s", g=GROUP),
                    )

            with tc.tile_pool(name="smx", bufs=1) as wp:
                negmax = wp.tile([TLOC, 1], f32, tag="negmax")
                nc.vector.reduce_max(
                    negmax, scores_sb, axis=mybir.AxisListType.X, negate=True
                )
                expt = wp.tile([TLOC, S], f32, tag="expt")
                sums = wp.tile([TLOC, 1], f32, tag="sums")
                nc.scalar.activation(
                    out=expt, in_=scores_sb, func=AF.Exp,
                    bias=negmax, scale=1.0, accum_out=sums,
                )
                rec = wp.tile([TLOC, 1], f32, tag="rec")
                nc.vector.reciprocal(rec, sums)
                outt = wp.tile([TLOC, S], f32, tag="outt")
                nc.vector.tensor_scalar_mul(outt, expt, rec)
                dma.dma_start(out=out_d[:, :], in_=outt)

    return nc


def _get_nc():
    if "nc" not in _CACHE:
        nc = _build()
        nc.finalize()  # Bacc legalization (wait splitting etc.) + freeze
        _CACHE["nc"] = nc
    return _CACHE["nc"]


def make_in_maps(decoder_outputs, encoder_outputs, mask, We, Wd, v):
    f32 = np.float32
    f16 = np.float16
    WeT = np.ascontiguousarray(We.T.astype(f16))
    WdT = np.ascontiguousarray(Wd.T.astype(f16))
    v4 = np.ascontiguousarray(v.astype(f16).reshape(D // 128, 128).T)

    in_maps = []
    for k in range(NCORES):
        b, th = k // 2, k % 2
        in_maps.append({
            "encT": np.ascontiguousarray(encoder_outputs[b].astype(f16).T),
            "decT": np.ascontiguousarray(
                decoder_outputs[b, th * TLOC:(th + 1) * TLOC].astype(f16).T
            ),
            "WeT": WeT,
            "WdT": WdT,
            "v4": v4,
            "maskbias": np.tile(
                np.where(mask[b], f32(-1e30), f32(0)).astype(f32), GROUP
            ).reshape(1, GROUP * S),
        })
    return in_maps


def assemble(results):
    full = np.empty((B, T, S), dtype=np.float32)
    for k in range(NCORES):
        b, th = k // 2, k % 2
        full[b, th * TLOC:(th + 1) * TLOC] = results[k]["out"]
    return full


def kernel(decoder_outputs, encoder_outputs, mask, We, Wd, v):
    from concourse.bass_utils import run_bass_kernel_spmd

    nc = _get_nc()
    in_maps = make_in_maps(decoder_outputs, encoder_outputs, mask, We, Wd, v)
    res = run_bass_kernel_spmd(nc, in_maps, core_ids=list(range(NCORES)))
    return assemble(res.results)


# revision 18
# speedup vs baseline: 4.2759x; 4.2759x over previous
"""Additive (Bahdanau) attention on 8 TRN2 NeuronCores.

scores[b,t,s] = softmax_s( sum_d v[d] * tanh(e1[b,s,d] + e2[b,t,d]) )  with mask
  e1 = enc @ We.T   [B,S,D]
  e2 = dec @ Wd.T   [B,T,D]

Sharding: pure data-parallel, core k handles batch b=k//2, t-half k%2
(128 t-rows each). No collectives.

v3 pipeline (per core):
- Mask compression: masked s-columns produce exactly 0 after softmax, so the
  host gathers only the ~Kp unmasked columns of enc^T (padded to a common Kp
  across cores; pads killed by a -1e30 bias before softmax) and scatters the
  device output back. Halves all per-s work.
- e1T[d,s']=We@encT_kept, e2T[d,t]=Wd@decT on TensorE (fp16, f32 PSUM).
- Main loop in super-groups of 8 t: DVE tensor_scalar_add (fp16 4x mode)
  stages e1T_j + e2T_j[:,t] into wide tiles; ScalarE runs ONE big tanh per
  (super-group, d-tile) amortizing the per-instruction bubble; TensorE
  matvec v_j.T @ act -> PSUM score rows; wide DVE add (+padbias) gathers 4
  PSUM banks -> SBUF row; DMA re-spreads rows onto scores partitions
  (compute engines can't write partitions !=0/32/64/96, DMA can).
- softmax: reduce_max(neg) -> Exp(bias=-max, accum_out=sums) -> recip -> mul.
"""

import numpy as np

B, T, S, D = 4, 256, 512, 512
C = 2 * D
NCORES = 8
TLOC = 128  # t-rows per core
GROUP = 4  # score rows gathered per DMA (4 PSUM banks)
SG = 8  # t-rows per ScalarE tanh batch

_CACHE = {}


def _build(kp, repeat=1):
    import concourse.mybir as mybir
    from concourse import bacc
    from concourse.tile import TileContext

    f32 = mybir.dt.float32
    f16 = mybir.dt.float16
    AF = mybir.ActivationFunctionType

    nc = bacc.Bacc()
    encT_d = nc.declare_dram_parameter("encT", [C, kp], f16, isOutput=False)
    decT_d = nc.declare_dram_parameter("decT", [D, TLOC], f16, isOutput=False)
    WeT_d = nc.declare_dram_parameter("WeT", [C, D], f16, isOutput=False)
    WdT_d = nc.declare_dram_parameter("WdT", [D, D], f16, isOutput=False)
    v4_d = nc.declare_dram_parameter("v4", [128, D // 128], f16, isOutput=False)
    mb_d = nc.declare_dram_parameter("padbias", [1, GROUP * kp], f32,
                                     isOutput=False)
    out_d = nc.declare_dram_parameter("out", [TLOC, kp], f32, isOutput=True)

    ND = D // 128  # 4 d-tiles
    NC_ = C // 128  # 8 c-tiles

    with TileContext(nc) as tc:
        with tc.tile_pool(name="persist", bufs=1) as pp:
            dma = nc.default_dma_engine

            WeT_sb = []
            encT_sb = []
            for ci in range(NC_):
                tw = pp.tile([128, D], f16, tag=f"WeT{ci}")
                dma.dma_start(out=tw, in_=WeT_d[ci * 128:(ci + 1) * 128, :])
                WeT_sb.append(tw)
                te = pp.tile([128, kp], f16, tag=f"encT{ci}")
                dma.dma_start(out=te, in_=encT_d[ci * 128:(ci + 1) * 128, :])
                encT_sb.append(te)
            WdT_sb = []
            decT_sb = []
            for di in range(ND):
                tw = pp.tile([128, D], f16, tag=f"WdT{di}")
                dma.dma_start(out=tw, in_=WdT_d[di * 128:(di + 1) * 128, :])
                WdT_sb.append(tw)
                td = pp.tile([128, TLOC], f16, tag=f"decT{di}")
                dma.dma_start(out=td, in_=decT_d[di * 128:(di + 1) * 128, :])
                decT_sb.append(td)
            v_sb = pp.tile([128, ND], f16, tag="v4")
            dma.dma_start(out=v_sb, in_=v4_d[:, :])
            mb_sb = pp.tile([1, GROUP * kp], f32, tag="mb_sb")
            dma.dma_start(out=mb_sb, in_=mb_d[:, :])

            # e1T[d,s'] and e2T[d,t], converted to fp16 for the DVE adds
            e1T_sb = []
            e2T_sb = []
            with tc.tile_pool(name="mm_psum", bufs=2, space="PSUM") as mmp:
                for dj in range(ND):
                    ps = mmp.tile([128, kp], f32, tag="pe1")
                    for ci in range(NC_):
                        nc.tensor.matmul(
                            ps,
                            WeT_sb[ci][:, dj * 128:(dj + 1) * 128],
                            encT_sb[ci],
                            start=(ci == 0),
                            stop=(ci == NC_ - 1),
                        )
                    t_ = pp.tile([128, kp], f16, tag=f"e1T{dj}")
                    nc.vector.tensor_copy(t_, ps)
                    e1T_sb.append(t_)
                for ej in range(ND):
                    ps = mmp.tile([128, TLOC], f32, tag="pe2")
                    for di in range(ND):
                        nc.tensor.matmul(
                            ps,
                            WdT_sb[di][:, ej * 128:(ej + 1) * 128],
                            decT_sb[di],
                            start=(di == 0),
                            stop=(di == ND - 1),
                        )
                    t_ = pp.tile([128, TLOC], f32, tag=f"e2T{ej}")
                    nc.vector.tensor_copy(t_, ps)
                    e2T_sb.append(t_)

            scores_sb = pp.tile([TLOC, kp], f32, tag="scores")
            with (
                tc.tile_pool(name="wide", bufs=2) as wpool,
                tc.tile_pool(name="acts", bufs=2) as apool,
                tc.tile_pool(name="stage", bufs=3) as sp,
                tc.tile_pool(name="row_psum", bufs=2, space="PSUM") as rp,
            ):
                for rep in range(repeat):
                    for sg in range(TLOC // SG):
                        t0 = sg * SG
                        acts = []
                        for j in range(ND):
                            wide = wpool.tile([128, SG, kp], f16,
                                              tag=f"wide{j}")
                            for u in range(SG):
                                t = t0 + u
                                nc.vector.tensor_scalar_add(
                                    wide[:, u, :], e1T_sb[j],
                                    e2T_sb[j][:, t:t + 1],
                                )
                            act = apool.tile([128, SG, kp], f16,
                                             tag=f"act{j}")
                            nc.scalar.activation(
                                out=act, in_=wide, func=AF.Tanh,
                            )
                            acts.append(act)
                        for half in range(SG // GROUP):
                            g = (t0 // GROUP) + half
                            rowg = rp.tile([1, GROUP, 512], f32, tag="rowg")
                            for i in range(GROUP):
                                u = half * GROUP + i
                                for j in range(ND):
                                    nc.tensor.matmul(
                                        rowg[0:1, i, 0:kp],
                                        v_sb[:, j:j + 1],
                                        acts[j][:, u, :],
                                        start=(j == 0), stop=(j == ND - 1),
                                    )
                            stage = sp.tile([1, GROUP * kp], f32, tag="stage")
                            nc.vector.tensor_tensor(
                                stage, rowg[0:1, :, 0:kp], mb_sb,
                                op=mybir.AluOpType.add,
                            )
                            dma.dma_start(
                                out=scores_sb[g * GROUP:(g + 1) * GROUP, :],
                                in_=stage.rearrange("p (g s) -> p g s",
                                                    g=GROUP),
                            )

            with tc.tile_pool(name="smx", bufs=1) as wp:
                negmax = wp.tile([TLOC, 1], f32, tag="negmax")
                nc.vector.reduce_max(
                    negmax, scores_sb, axis=mybir.AxisListType.X, negate=True
                )
                expt = wp.tile([TLOC, kp], f32, tag="expt")
                sums = wp.tile([TLOC, 1], f32, tag="sums")
                nc.scalar.activation(
                    out=expt, in_=scores_sb, func=AF.Exp,
                    bias=negmax, scale=1.0, accum_out=sums,
                )
                rec = wp.tile([TLOC, 1], f32, tag="rec")
                nc.vector.reciprocal(rec, sums)
                outt = wp.tile([TLOC, kp], f32, tag="outt")
                nc.vector.tensor_scalar_mul(outt, expt, rec)
                dma.dma_start(out=out_d[:, :], in_=outt)

    return nc


def _get_nc(kp, repeat=1):
    key = ("nc", kp, repeat)
    if key not in _CACHE:
        nc = _build(kp, repeat)
        nc.finalize()  # Bacc legalization (wait splitting etc.) + freeze
        _CACHE[key] = nc
    return _CACHE[key]


def make_in_maps(decoder_outputs, encoder_outputs, mask, We, Wd, v):
    f32 = np.float32
    f16 = np.float16
    mask = np.asarray(mask)
    keep_idx = [np.where(~mask[b])[0] for b in range(B)]
    nkeep = [len(ix) for ix in keep_idx]
    kp = max(16, -16 * (-max(nkeep) // 16))  # round up to multiple of 16

    WeT = np.ascontiguousarray(We.T.astype(f16))
    WdT = np.ascontiguousarray(Wd.T.astype(f16))
    v4 = np.ascontiguousarray(v.astype(f16).reshape(D // 128, 128).T)

    in_maps = []
    for k in range(NCORES):
        b, th = k // 2, k % 2
        ix = keep_idx[b]
        ix_pad = np.concatenate(
            [ix, np.full(kp - len(ix), ix[-1], dtype=ix.dtype)]
        )
        encT_kept = np.ascontiguousarray(
            encoder_outputs[b].astype(f16).T[:, ix_pad]
        )
        pad = np.concatenate(
            [np.zeros(len(ix), f32), np.full(kp - len(ix), f32(-1e30))]
        )
        in_maps.append({
            "encT": encT_kept,
            "decT": np.ascontiguousarray(
                decoder_outputs[b, th * TLOC:(th + 1) * TLOC].astype(f16).T
            ),
            "WeT": WeT,
            "WdT": WdT,
            "v4": v4,
            "padbias": np.tile(pad, GROUP).reshape(1, GROUP * kp),
        })
    meta = {"kp": kp, "keep_idx": keep_idx, "nkeep": nkeep}
    return in_maps, meta


def assemble(results, meta):
    full = np.zeros((B, T, S), dtype=np.float32)
    for k in range(NCORES):
        b, th = k // 2, k % 2
        ix = meta["keep_idx"][b]
        out = results[k]["out"]  # [TLOC, kp]
        full[b, th * TLOC:(th + 1) * TLOC, ix] = out[:, :len(ix)].T
    return full


def kernel(decoder_outputs, encoder_outputs, mask, We, Wd, v):
    from concourse.bass_utils import run_bass_kernel_spmd

    in_maps, meta = make_in_maps(
        decoder_outputs, encoder_outputs, mask, We, Wd, v
    )
    nc = _get_nc(meta["kp"])
    res = run_bass_kernel_spmd(nc, in_maps, core_ids=list(range(NCORES)))
    return assemble(res.results, meta)
